# revision 25
# baseline (speedup 1.0000x reference)
"""Trainium2 Bass kernel for nn_ClassificationLoss (BCE-with-logits + graph
Laplacian regularizer), data-parallel over 8 NeuronCores.

loss = mean(softplus(logits) - targets*logits)
       + 1e-4 * 0.5 * sum_e ||params[parent_e] - params[child_e]||^2

Strategy (per core c of 8), all tensors bf16 (keeps DVE 2x mode):
  - Regularizer: edges [2500c, 2500c+2500) padded to 2560; params rows
    fetched with 8 single-packet dma_gather calls (4 x 640 idxs per endpoint,
    round-robin over 4 SWDGE queues so descriptor generation and ring drains
    overlap; 40 descriptors/engine per call stays under the 64-desc packet
    ceiling). DVE subtract + DVE fused multiply-reduce for sum((p-c)^2).
  - BCE: rows [256c, 256c+256) streamed in [128 x 5000] chunks on the two
    HWDGE rings. softplus = Ln(bias=1) o Exp on ACT; all Exp passes issued
    before all Ln passes so only 2 activation-table loads happen (the
    exp-table / ln-table alternation in the naive order costs 8 loads).
    sum(t*x) via one fused DVE tensor_tensor_reduce pass per chunk.
  - Each core writes a [128, 16] f32 partial-sum tensor; host reduces in f64.
"""
import os
import sys

import numpy as np
import ml_dtypes

for _p in ("/opt/trn_rl_repo", "/root/.axon_site/_ro/trn_rl_repo"):
    if os.path.isdir(_p) and _p not in sys.path:
        sys.path.append(_p)

from contextlib import ExitStack

import concourse.bass as bass
import concourse.tile as tile
from concourse import bacc, mybir
from concourse.bass_utils import run_bass_kernel_spmd
from concourse.library_config import mlp

bf16 = ml_dtypes.bfloat16
AF = mybir.ActivationFunctionType
ALU = mybir.AluOpType

# build-config knobs (env-overridable for bisection)
SINGLE_PACKET = os.environ.get("K_SP", "1") == "1"
NQ = int(os.environ.get("K_NQ", "4"))
FUSED = os.environ.get("K_FUSED", "1") == "1"
SER = os.environ.get("K_SER", "0") == "1"   # bufs=1 pools: latency proxy

N_CORES = 8
BATCH, N_LABELS, HIDDEN, N_EDGES = 2048, 10000, 768, 20000
PENALTY = 1e-4
ROWS = BATCH // N_CORES            # 256 rows per core
BLOCKS = ROWS // 128               # 2 partition blocks
NCH = 2                            # bce col-chunks per block
CHUNK = N_LABELS // NCH            # 5000
EDGES_PC = N_EDGES // N_CORES      # 2500 edges per core
EDGES_PAD = 2560                   # padded to 4*640
GCALLS = 4                         # gather calls per endpoint
GIDX = EDGES_PAD // GCALLS         # 640 idxs per gather call
GCOLS = EDGES_PAD // 128           # 20 gather cols
RCH = GCALLS                       # reg chunks, aligned 1:1 with gather calls
RCOLS = GCOLS // RCH               # 5 cols per reg chunk
NBCE = BLOCKS * NCH                # 4 bce chunks
# partials columns: [0:4) softplus sums, [4:8) t*x sums, [8:12) reg sums
P_COLS = 16

_cache = {}


def _build_nc(reps=1):
    nc = bacc.Bacc("TRN2", target_bir_lowering=False, debug=False,
                   num_devices=N_CORES, num_swdge_queues=NQ)
    with tile.TileContext(nc) as tc, ExitStack() as ctx:
        nb = 1 if SER else 2
        io_pool = ctx.enter_context(tc.tile_pool(name="io", bufs=nb))
        ex_pool = ctx.enter_context(tc.tile_pool(name="ex", bufs=NBCE))
        act_pool = ctx.enter_context(tc.tile_pool(name="act", bufs=nb))
        g_pool = ctx.enter_context(tc.tile_pool(name="g", bufs=1))
        d_pool = ctx.enter_context(tc.tile_pool(name="d", bufs=nb))

        logits_d = nc.dram_tensor(
            "logits", [BLOCKS, 128, N_LABELS], mybir.dt.bfloat16,
            kind="ExternalInput")
        targets_d = nc.dram_tensor(
            "targets", [BLOCKS, 128, N_LABELS], mybir.dt.bfloat16,
            kind="ExternalInput")
        params_d = nc.dram_tensor(
            "params", [N_LABELS, HIDDEN], mybir.dt.bfloat16,
            kind="ExternalInput")
        idx_d = nc.dram_tensor(
            "idx", [128, 2 * EDGES_PAD // 16], mybir.dt.int16,
            kind="ExternalInput")
        out_d = nc.dram_tensor(
            "partials", [128, P_COLS], mybir.dt.float32, kind="ExternalOutput")

        parts = g_pool.tile([128, P_COLS], mybir.dt.float32)
        nc.vector.memset(parts[:], 0.0)

        with tc.tile_critical():
            nc.gpsimd.load_library(mlp)

        # --- regularizer gathers: 8 single-packet calls over 4 SWDGE queues
        it_all = g_pool.tile([128, 2 * EDGES_PAD // 16], mybir.dt.int16)
        nc.sync.dma_start(out=it_all[:], in_=idx_d[:])
        itp = it_all[:, :EDGES_PAD // 16]
        itc = it_all[:, EDGES_PAD // 16:]
        seg = RCOLS * HIDDEN
        gp_t = [g_pool.tile([128, seg], mybir.dt.bfloat16, name=f"gp{q}")
                for q in range(GCALLS)]
        gc_t = [g_pool.tile([128, seg], mybir.dt.bfloat16, name=f"gc{q}")
                for q in range(GCALLS)]

        def emit_body():
            gq = 0
            for idx_t, dst_l in ((itp, gp_t), (itc, gc_t)):
                for q in range(GCALLS):
                    sl_i = slice(q * (GIDX // 16), (q + 1) * (GIDX // 16))
                    nc.gpsimd.dma_gather(
                        dst_l[q][:].rearrange("p (c s) -> p c s", s=HIDDEN),
                        params_d[:], idx_t[:, sl_i], GIDX, GIDX, HIDDEN,
                        single_packet=SINGLE_PACKET, queue_num=gq % NQ)
                    gq += 1

            # --- BCE: DMA + Exp + fused t*x per chunk; Ln batched after
            ex_tiles = []
            col = 0
            for b in range(BLOCKS):
                for j in range(NCH):
                    sl = slice(j * CHUNK, (j + 1) * CHUNK)
                    lt = io_pool.tile([128, CHUNK], mybir.dt.bfloat16, tag="lt")
                    nc.sync.dma_start(out=lt[:], in_=logits_d[b, :, sl])
                    tt = io_pool.tile([128, CHUNK], mybir.dt.bfloat16, tag="tt")
                    # second HWDGE ring (ACT) so both streams drain in parallel
                    nc.scalar.dma_start(out=tt[:], in_=targets_d[b, :, sl])
                    ex = ex_pool.tile([128, CHUNK], mybir.dt.bfloat16, tag="ex")
                    nc.scalar.activation(out=ex[:], in_=lt[:], func=AF.Exp)
                    ex_tiles.append(ex)
                    tx = act_pool.tile([128, CHUNK], mybir.dt.bfloat16,
                                       tag="sink")
                    acc = parts[:, NBCE + col:NBCE + col + 1]
                    if FUSED:
                        nc.vector.scalar_tensor_tensor(
                            out=tx[:], in0=lt[:], scalar=1.0, in1=tt[:],
                            op0=ALU.mult, op1=ALU.mult, accum_out=acc)
                    else:
                        nc.vector.tensor_tensor(out=tx[:], in0=lt[:],
                                                in1=tt[:], op=ALU.mult)
                        nc.vector.reduce_sum(out=acc, in_=tx[:],
                                             axis=mybir.AxisListType.X)
                    col += 1

            for col, ex in enumerate(ex_tiles):
                sp = act_pool.tile([128, CHUNK], mybir.dt.bfloat16, tag="sink")
                nc.scalar.activation(out=sp[:], in_=ex[:], func=AF.Ln,
                                     bias=1.0, accum_out=parts[:, col:col + 1])

            # --- regularizer: subtract + fused square-reduce, all on DVE
            for r in range(RCH):
                d = d_pool.tile([128, seg], mybir.dt.bfloat16, tag="d")
                nc.vector.tensor_tensor(out=d[:], in0=gp_t[r][:],
                                        in1=gc_t[r][:], op=ALU.subtract)
                d2 = d_pool.tile([128, seg], mybir.dt.bfloat16, tag="d2")
                acc = parts[:, 2 * NBCE + r:2 * NBCE + r + 1]
                if FUSED:
                    nc.vector.scalar_tensor_tensor(
                        out=d2[:], in0=d[:], scalar=1.0, in1=d[:],
                        op0=ALU.mult, op1=ALU.mult, accum_out=acc)
                else:
                    nc.vector.tensor_tensor(out=d2[:], in0=d[:], in1=d[:],
                                            op=ALU.mult)
                    nc.vector.reduce_sum(out=acc, in_=d2[:],
                                         axis=mybir.AxisListType.X)

        for _ in range(reps):
            emit_body()

        nc.sync.dma_start(out=out_d[:], in_=parts[:])
    nc.compile()
    return nc


def _wrap_idxs(idxs):
    """[N] ints -> [128, N/16] int16 dma_gather layout: idx i at [i%16, i//16],
    rows replicated 8x down the 128 partitions."""
    n = idxs.size
    a = np.zeros((16, n // 16), np.int16)
    a[np.arange(n) % 16, np.arange(n) // 16] = idxs.astype(np.int16)
    return np.tile(a, (8, 1))


def _get_nc():
    if "nc" not in _cache:
        _cache["nc"] = _build_nc()
    return _cache["nc"]


def make_in_maps(logits, targets, params, parent_idx, child_idx):
    lb = logits.astype(bf16).reshape(N_CORES, BLOCKS, 128, N_LABELS)
    tb = targets.astype(bf16).reshape(N_CORES, BLOCKS, 128, N_LABELS)
    pb = params.astype(bf16)
    in_maps = []
    for c in range(N_CORES):
        pe = parent_idx[c * EDGES_PC:(c + 1) * EDGES_PC].astype(np.int64)
        ce = child_idx[c * EDGES_PC:(c + 1) * EDGES_PC].astype(np.int64)
        order = np.argsort(pe, kind="stable")  # HBM locality for parent gather
        pe, ce = pe[order], ce[order]
        pad = EDGES_PAD - EDGES_PC
        pe = np.concatenate([pe, np.zeros(pad, np.int64)])
        ce = np.concatenate([ce, np.zeros(pad, np.int64)])
        in_maps.append({
            "logits": lb[c], "targets": tb[c], "params": pb,
            "idx": np.concatenate([_wrap_idxs(pe), _wrap_idxs(ce)], axis=1),
        })
    return in_maps


def reduce_partials(partials_list):
    p = np.stack([np.asarray(x, dtype=np.float64) for x in partials_list])
    sp_sum = p[:, :, 0:NBCE].sum()
    tx_sum = p[:, :, NBCE:2 * NBCE].sum()
    reg_sum = p[:, :, 2 * NBCE:2 * NBCE + RCH].sum()
    bce = (sp_sum - tx_sum) / (BATCH * N_LABELS)
    loss = bce + PENALTY * 0.5 * reg_sum
    return np.asarray(loss, dtype=np.float32)


def kernel(logits, targets, params, parent_idx, child_idx):
    nc = _get_nc()
    in_maps = make_in_maps(logits, targets, params, parent_idx, child_idx)
    res = run_bass_kernel_spmd(nc, in_maps, list(range(N_CORES)))
    return reduce_partials([r["partials"] for r in res.results])


if __name__ == "__main__":
    rng = np.random.default_rng(0)
    out = kernel(
        rng.standard_normal((BATCH, N_LABELS)).astype(np.float32),
        rng.random((BATCH, N_LABELS)).astype(np.float32),
        rng.standard_normal((N_LABELS, HIDDEN)).astype(np.float32),
        rng.integers(0, N_LABELS, N_EDGES).astype(np.int32),
        rng.integers(0, N_LABELS, N_EDGES).astype(np.int32),
    )
    print("loss:", out, out.shape, out.dtype)
